# revision 29
# baseline (speedup 1.0000x reference)
"""CAM (channel attention module) kernel for Trainium2, 8-core SPMD.

Problem: x (16, 512, 64, 64) f32, gamma (1,) f32.
  v = x.reshape(B, C, N);  E = v @ v.T  (B x 512 x 512)
  att = softmax(rowmax(E) - E)  ==  exp(rowmin(E) - E) / rowsum(...)
  out = gamma * (att @ v) + x

Sharding: data-parallel over batch, 2 batches per core, no collectives.

Mixed-precision plan (all accumulation in f32 PSUM):
  inputs (host-prepared): x16 = fp16(x)  [energy operands]
                          xq  = (hi, lo) fp8-e4m3 pair with hi+lo == x to
                                ~0.15% (v operand + residual of the
                                output matmul)
  vT: loaded straight from DRAM with the DMA xbar transpose (fp16) in 9
      n-chunks (first two halved for a faster cold start) -- no PE
      transposes and no PSUM->SBUF copies.
  E:  energy = vT.T @ vT per 128-row tile, fp16 operands, f32 PSUM,
      n-chunk-outer so matmuls start on the first xbar chunk.
      Symmetry: row-tile ct computes only columns >= ct*128; the 6 missing
      blocks are mirrored from earlier rows by PE transpose into PSUM.
  S:  softmax of (rowmin(E) - E) [stable form of softmax(rowmax-E)]:
      DVE rowmin, ACT exp (fp16 out) with accumulated rowsum, DVE
      reciprocal; att = fp16(gamma/rowsum * exp) + IDENTITY on the
      diagonal block, so the output matmul computes gamma*att@v + v
      directly (no epilogue add; gamma==0 -> y = hi+lo = x to 1.4e-3).
  A:  PE transposes att -> attT per row-tile (fp16 -- walrus rejects fp8
      transposes), one DVE copy quantizes PSUM -> fp8 SBUF.
  O:  out = sum_dt attT8[dt].T @ (hi[dt], lo[dt]) as fp8 DoubleRow
      matmuls; the lhsT broadcasts (stride 0) over the 2 k-slots, the rhs
      packs the interleaved (hi, lo) planes. Half the fp16 cycle cost at
      fp16-class accuracy. DVE/ACT convert PSUM->fp16 staging, paired
      fp16 stores. y upcast to f32 on host.

Scheduling notes (CoreSim v1 cost model): DMA transfer time is charged
to the issuing engine's queue and the tile scheduler chains
simultaneously-ready DMAs across queues in scheduled order (lane-tick
semaphores), so all DMAs go on the SP queue in a deliberate order:
gamma, all loads (batch 0 then batch 1), then data-dependent stores.
emit_A runs before epilogue(ct+1) so the attT copy beats the next
softmax chain into the DVE queue. Span 85.2us: SP(DMA) 93% busy is the
binding engine; PE 76%.
"""
import sys

import ml_dtypes
import numpy as np

if "/opt/trn_rl_repo" not in sys.path:
    sys.path.insert(0, "/opt/trn_rl_repo")

import concourse.bass as bass
import concourse.tile as tile
from concourse import bacc, mybir
from concourse.bass_utils import run_bass_kernel_spmd
from concourse.masks import make_identity

N_CORES = 8
B_FULL = 16
B_PER_CORE = B_FULL // N_CORES  # 2
C = 512            # channels
HW = 4096          # H*W
CT = C // 128      # 4 channel tiles
NCH = HW // 512    # 8 output column chunks
QW = HW // 4       # quarter of H*W (xbar-transpose granularity)

f32 = mybir.dt.float32
f16 = mybir.dt.float16
f8 = mybir.dt.float8e4
F8NP = ml_dtypes.float8_e4m3

_CACHE = {}


def _build_nc(reps: int = 1):
    nc = bacc.Bacc(None, target_bir_lowering=False)
    x16_d = nc.dram_tensor("x16", [B_PER_CORE, C, HW], f16, kind="ExternalInput")
    xq_d = nc.dram_tensor("xq", [B_PER_CORE, C, 2, HW], f8, kind="ExternalInput")
    g_d = nc.dram_tensor("gamma", [1], f32, kind="ExternalInput")
    y_d = nc.dram_tensor("y", [B_PER_CORE, C, HW], f16, kind="ExternalOutput")

    with tile.TileContext(nc) as tc:
        with (
            tc.tile_pool(name="pvt", bufs=2) as pvt,        # vT fp16 quarters
            tc.tile_pool(name="pv8", bufs=2) as pv8,        # (hi,lo) fp8 rows
            tc.tile_pool(name="pa16", bufs=1) as pa16,      # exp fp16
            tc.tile_pool(name="pa8", bufs=1) as pa8,        # att fp8
            tc.tile_pool(name="paT", bufs=1) as paT,        # attT fp8 (dup'd)
            tc.tile_pool(name="pmir", bufs=1) as pmir,      # mirror blocks
            tc.tile_pool(name="pstage", bufs=10) as pstage, # out staging
            tc.tile_pool(name="psmall", bufs=4) as psmall,  # per-ct scalars
            tc.tile_pool(name="psing", bufs=1) as psing,    # ident, gamma
            tc.tile_pool(name="pep", bufs=1, space="PSUM") as pep,
            tc.tile_pool(name="pop", bufs=4, space="PSUM") as pop,
        ):
            identf = psing.tile([128, 128], f32)
            make_identity(nc, identf)
            ident16 = psing.tile([128, 128], f16)
            nc.vector.tensor_copy(out=ident16, in_=identf)
            gam = psing.tile([128, 1], f32)

            def load_gamma():
                # on sync, emitted before everything: a SWDGE gamma load
                # gets ring-chained mid-stream and opens a ~4us hole
                g_ap = g_d[:]
                nc.sync.dma_start(
                    out=gam,
                    in_=bass.AP(tensor=g_ap.tensor, offset=g_ap.offset,
                                ap=[[0, 128], [1, 1]]),
                )

            # n-chunk widths (in 128-wide k-tiles) for the xbar transposes;
            # the first two are halved so the energy matmuls start sooner
            CHUNKS = [2, 2, 4, 4, 4, 4, 4, 4, 4]
            assert sum(CHUNKS) == 32

            def load_batch(b, bi):
                # vT chunk-tiles via DMA xbar transpose (SP queue):
                # vt[g][p, kk, c] = x16[b, c, off + kk*128 + p]
                vt = []
                off = 0
                for g, w in enumerate(CHUNKS):
                    t_ = pvt.tile([128, w, C], f16, tag=f"vt{g}",
                                  name=f"vt{g}")
                    nc.sync.dma_start_transpose(
                        out=t_[:],
                        in_=x16_d[b, :, off * 128:(off + w) * 128],
                    )
                    vt.append(t_)
                    off += w
                # (hi, lo) fp8 pair per channel row-tile, behind the xbars
                # on the same SP queue (simultaneously-ready DMAs are
                # chain-serialized across engines anyway; keeping one
                # queue avoids cross-engine handoff holes)
                v8 = [pv8.tile([128, 2, HW], f8, tag=f"v8{ct}", name=f"v8{ct}")
                      for ct in range(CT)]
                for ct in range(CT):
                    nc.sync.dma_start(
                        out=v8[ct],
                        in_=xq_d[b, ct * 128:(ct + 1) * 128, :, :],
                    )
                return vt, v8

            batches = [bb for _ in range(reps) for bb in range(B_PER_CORE)]
            # gamma first, then all loads, before any stores: the DMA
            # ring chains simultaneously-ready DMAs in scheduled order,
            # so late/slow DMAs in front of the chain stall everything.
            load_gamma()
            loads = [load_batch(b, bi) for bi, b in enumerate(batches)]

            for bi, b in enumerate(batches):
                vt, v8 = loads[bi]

                # ---- E: energy (fp16 operands, f32 accum), quarter-outer
                # so the first matmuls start as soon as xbar quarter 0 lands.
                ep = [pep.tile([128, C], f32, tag=f"ep{ct}", name=f"ep{ct}")
                      for ct in range(CT)]
                NG = len(CHUNKS)
                # chunk-outer for the head (E starts on the first xbar
                # chunk), then ct-outer for the last chunks so the four
                # row-tiles STOP staggered and their softmax chains
                # pipeline on DVE/ACT instead of piling up
                TAIL = 0

                def e_mm(g, ct, kk):
                    off = ct * 128
                    nc.tensor.matmul(
                        ep[ct][:, off:],
                        lhsT=vt[g][:, kk, off:off + 128],
                        rhs=vt[g][:, kk, off:],
                        start=(g == 0 and kk == 0),
                        stop=(g == NG - 1 and kk == CHUNKS[g] - 1),
                    )

                for g in range(NG - TAIL):
                    for ct in range(CT):
                        for kk in range(CHUNKS[g]):
                            e_mm(g, ct, kk)
                for ct in range(CT):
                    for g in range(NG - TAIL, NG):
                        for kk in range(CHUNKS[g]):
                            e_mm(g, ct, kk)
                # ---- S: fused softmax -> fp8 att (+ identity on diagonal)
                mirror_sb = {}
                a8s = [None] * CT

                def epilogue(ct):
                    off = ct * 128
                    # stash blocks that later rows mirror
                    for dst in range(ct + 1, CT):
                        sb = pmir.tile([128, 128], f32, tag=f"m{dst}{ct}")
                        nc.vector.tensor_copy(
                            out=sb, in_=ep[ct][:, dst * 128:(dst + 1) * 128],
                        )
                        mirror_sb[(dst, ct)] = sb
                    for dt in range(ct):
                        nc.tensor.transpose(
                            ep[ct][:, dt * 128:(dt + 1) * 128],
                            mirror_sb[(ct, dt)], identf,
                        )
                    mn = psmall.tile([128, 1], f32, tag="mn")
                    nc.vector.tensor_reduce(
                        out=mn, in_=ep[ct], axis=mybir.AxisListType.X,
                        op=mybir.AluOpType.min,
                    )
                    a16 = pa16.tile([128, C], f16, tag=f"a16_{ct % 2}")
                    ss = psmall.tile([128, 1], f32, tag="ss")
                    nc.scalar.activation(
                        out=a16, in_=ep[ct],
                        func=mybir.ActivationFunctionType.Exp,
                        bias=mn, scale=-1.0, accum_out=ss,
                    )
                    rg = psmall.tile([128, 1], f32, tag="rg")
                    nc.vector.reciprocal(out=rg, in_=ss)
                    nc.vector.tensor_mul(out=rg, in0=rg, in1=gam)
                    ag = pa8.tile([128, C], f16, tag=f"ag_{ct}")
                    nc.vector.tensor_scalar_mul(ag, a16, rg)
                    # identity fold: att' = gamma*att + I, so O emits
                    # gamma*att@v + v directly (on DVE right after the
                    # scale: avoids two cross-engine sem hops)
                    nc.vector.tensor_add(
                        out=ag[:, off:off + 128], in0=ag[:, off:off + 128],
                        in1=ident16,
                    )
                    a8s[ct] = ag

                # ---- A + O per row-tile; epilogue(ct+1) is emitted before
                # O(ct) so the softmax chain of the next tile overlaps the
                # current tile's output matmuls.
                def emit_A(ct):
                    # transpose in fp16 (fp8 PE transpose is rejected by
                    # walrus); quantize to fp8 in one PSUM->SBUF copy; the
                    # DoubleRow lhsT broadcasts it over the 2 k-slots
                    atp = pop.tile([128, CT, 128], f16, tag="op")
                    for dt in range(CT):
                        nc.tensor.transpose(
                            atp[:, dt, :],
                            a8s[ct][:, dt * 128:(dt + 1) * 128], ident16,
                        )
                    aT = paT.tile([128, CT, 128], f8, tag=f"aT{ct}")
                    nc.vector.tensor_copy(out=aT, in_=atp)
                    return aT

                def emit_O(ct, aT):
                    for np_ in range(NCH // 2):  # paired n-chunks per store
                        st = pstage.tile([128, 2, 512], f16, tag="st")
                        for half in range(2):
                            n = np_ * 2 + half
                            op = pop.tile([128, 512], f32, tag="op")
                            for dt in range(CT):
                                nc.tensor.matmul(
                                    op,
                                    lhsT=aT[:, dt, :].unsqueeze(1)
                                        .broadcast_to((128, 2, 128)),
                                    rhs=v8[dt][:, :, n * 512:(n + 1) * 512],
                                    start=(dt == 0),
                                    stop=(dt == CT - 1),
                                    perf_mode=mybir.MatmulPerfMode.DoubleRow,
                                )
                            if half == 0:
                                nc.vector.tensor_copy(out=st[:, half], in_=op)
                            else:
                                nc.scalar.copy(out=st[:, half], in_=op)
                        # single DMA queue (SP): cross-engine DMA ring
                        # chaining creates stall holes, so keep the ring
                        # order = SP program order
                        nc.sync.dma_start(
                            out=y_d[b, ct * 128:(ct + 1) * 128,
                                    np_ * 1024:(np_ + 1) * 1024],
                            in_=st,
                        )

                # A(ct) is emitted before epilogue(ct+1) so the attT dup
                # copies beat the next softmax chain into the DVE/ACT
                # queues (O(ct) start depends on the dup copies).
                epilogue(0)
                aTs = {}
                for ct in range(CT):
                    aTs[ct] = emit_A(ct)
                    if ct + 1 < CT:
                        epilogue(ct + 1)
                    emit_O(ct, aTs[ct])

    nc.compile()
    return nc


def host_prep(x: np.ndarray):
    """x (B, C, HW) f32 -> (x16 fp16, xq fp8-pair [B, C, 2, HW])."""
    x16 = x.astype(np.float16)
    hi = x.astype(F8NP)
    lo = (x - hi.astype(np.float32)).astype(F8NP)
    xq = np.stack([hi, lo], axis=2)
    return x16, xq


def kernel(x: np.ndarray, gamma: np.ndarray) -> np.ndarray:
    x = np.ascontiguousarray(np.asarray(x, dtype=np.float32))
    gamma = np.ascontiguousarray(np.asarray(gamma, dtype=np.float32))
    B, Cc, H, W = x.shape
    xv = x.reshape(B, Cc, H * W)
    x16, xq = host_prep(xv)

    if "nc" not in _CACHE:
        _CACHE["nc"] = _build_nc()
    nc = _CACHE["nc"]

    in_maps = [
        {
            "x16": x16[i * B_PER_CORE:(i + 1) * B_PER_CORE],
            "xq": xq[i * B_PER_CORE:(i + 1) * B_PER_CORE],
            "gamma": gamma,
        }
        for i in range(N_CORES)
    ]
    res = run_bass_kernel_spmd(nc, in_maps, list(range(N_CORES)))
    y = np.concatenate([res.results[i]["y"] for i in range(N_CORES)], axis=0)
    return y.astype(np.float32).reshape(B, Cc, H, W)


# revision 33
# speedup vs baseline: 1.0038x; 1.0038x over previous
"""CAM (channel attention module) kernel for Trainium2, 8-core SPMD.

Problem: x (16, 512, 64, 64) f32, gamma (1,) f32.
  v = x.reshape(B, C, N);  E = v @ v.T  (B x 512 x 512)
  att = softmax(rowmax(E) - E)  ==  exp(rowmin(E) - E) / rowsum(...)
  out = gamma * (att @ v) + x

Sharding: data-parallel over batch, 2 batches per core, no collectives.

Mixed-precision plan (all accumulation in f32 PSUM):
  inputs (host-prepared): x16 = fp16(x)  [energy operands]
                          xq  = (hi, lo) fp8-e4m3 pair with hi+lo == x to
                                ~0.15% (v operand + residual of the
                                output matmul)
  vT: loaded straight from DRAM with the DMA xbar transpose (fp16) in 9
      n-chunks (first two halved for a faster cold start) -- no PE
      transposes and no PSUM->SBUF copies.
  E:  energy = vT.T @ vT per 128-row tile, fp16 operands, f32 PSUM,
      n-chunk-outer so matmuls start on the first xbar chunk.
      Symmetry: row-tile ct computes only columns >= ct*128; the 6 missing
      blocks are mirrored from earlier rows by PE transpose into PSUM.
  S:  softmax of (rowmin(E) - E) [stable form of softmax(rowmax-E)]:
      DVE rowmin, ACT exp (fp16 out) with accumulated rowsum, DVE
      reciprocal; att = fp16(gamma/rowsum * exp) + IDENTITY on the
      diagonal block, so the output matmul computes gamma*att@v + v
      directly (no epilogue add; gamma==0 -> y = hi+lo = x to 1.4e-3).
  A:  PE transposes att -> attT per row-tile (fp16 -- walrus rejects fp8
      transposes), one DVE copy quantizes PSUM -> fp8 SBUF.
  O:  out = sum_dt attT8[dt].T @ (hi[dt], lo[dt]) as fp8 DoubleRow
      matmuls; the lhsT broadcasts (stride 0) over the 2 k-slots, the rhs
      packs the interleaved (hi, lo) planes. Half the fp16 cycle cost at
      fp16-class accuracy. DVE/ACT convert PSUM->fp16 staging, paired
      fp16 stores. y upcast to f32 on host.

Scheduling notes (CoreSim v1 cost model): DMA transfer time is charged
to the issuing engine's queue and the tile scheduler chains
simultaneously-ready DMAs across queues in scheduled order (lane-tick
semaphores), so all DMAs go on the SP queue in a deliberate order:
gamma, all loads (batch 0 then batch 1), then data-dependent stores.
emit_A runs before epilogue(ct+1) so the attT copy beats the next
softmax chain into the DVE queue. Span 85.2us: SP(DMA) 93% busy is the
binding engine; PE 76%.
"""
import sys

import ml_dtypes
import numpy as np

if "/opt/trn_rl_repo" not in sys.path:
    sys.path.insert(0, "/opt/trn_rl_repo")

import concourse.bass as bass
import concourse.tile as tile
from concourse import bacc, mybir
from concourse.bass_utils import run_bass_kernel_spmd
from concourse.masks import make_identity

N_CORES = 8
B_FULL = 16
B_PER_CORE = B_FULL // N_CORES  # 2
C = 512            # channels
HW = 4096          # H*W
CT = C // 128      # 4 channel tiles
NCH = HW // 512    # 8 output column chunks
QW = HW // 4       # quarter of H*W (xbar-transpose granularity)

f32 = mybir.dt.float32
f16 = mybir.dt.float16
f8 = mybir.dt.float8e4
F8NP = ml_dtypes.float8_e4m3

_CACHE = {}


def _build_nc(reps: int = 1):
    nc = bacc.Bacc(None, target_bir_lowering=False)
    x16_d = nc.dram_tensor("x16", [B_PER_CORE, C, HW], f16, kind="ExternalInput")
    xq_d = nc.dram_tensor("xq", [B_PER_CORE, C, 2, HW], f8, kind="ExternalInput")
    g_d = nc.dram_tensor("gamma", [1], f32, kind="ExternalInput")
    y_d = nc.dram_tensor("y", [B_PER_CORE, C, HW], f16, kind="ExternalOutput")

    with tile.TileContext(nc) as tc:
        with (
            tc.tile_pool(name="pvt", bufs=2) as pvt,        # vT fp16 quarters
            tc.tile_pool(name="pv8", bufs=2) as pv8,        # (hi,lo) fp8 rows
            tc.tile_pool(name="pa16", bufs=1) as pa16,      # exp fp16
            tc.tile_pool(name="pa8", bufs=1) as pa8,        # att fp8
            tc.tile_pool(name="paT", bufs=1) as paT,        # attT fp8 (dup'd)
            tc.tile_pool(name="pmir", bufs=1) as pmir,      # mirror blocks
            tc.tile_pool(name="pstage", bufs=10) as pstage, # out staging
            tc.tile_pool(name="psmall", bufs=4) as psmall,  # per-ct scalars
            tc.tile_pool(name="psing", bufs=1) as psing,    # ident, gamma
            tc.tile_pool(name="pep", bufs=1, space="PSUM") as pep,
            tc.tile_pool(name="pop", bufs=4, space="PSUM") as pop,
        ):
            identf = psing.tile([128, 128], f32)
            make_identity(nc, identf)
            ident16 = psing.tile([128, 128], f16)
            nc.vector.tensor_copy(out=ident16, in_=identf)
            gam = psing.tile([128, 1], f32)

            def load_gamma():
                # on sync, emitted before everything: a SWDGE gamma load
                # gets ring-chained mid-stream and opens a ~4us hole
                g_ap = g_d[:]
                nc.sync.dma_start(
                    out=gam,
                    in_=bass.AP(tensor=g_ap.tensor, offset=g_ap.offset,
                                ap=[[0, 128], [1, 1]]),
                )

            # n-chunk widths (in 128-wide k-tiles) for the xbar transposes;
            # the first two are halved so the energy matmuls start sooner
            CHUNKS = [1, 1, 2, 4, 4, 4, 4, 4, 4, 4]
            assert sum(CHUNKS) == 32

            def load_batch(b, bi):
                # vT chunk-tiles via DMA xbar transpose (SP queue):
                # vt[g][p, kk, c] = x16[b, c, off + kk*128 + p]
                vt = []
                off = 0
                for g, w in enumerate(CHUNKS):
                    t_ = pvt.tile([128, w, C], f16, tag=f"vt{g}",
                                  name=f"vt{g}")
                    nc.sync.dma_start_transpose(
                        out=t_[:],
                        in_=x16_d[b, :, off * 128:(off + w) * 128],
                    )
                    vt.append(t_)
                    off += w
                # (hi, lo) fp8 pair per channel row-tile, behind the xbars
                # on the same SP queue (simultaneously-ready DMAs are
                # chain-serialized across engines anyway; keeping one
                # queue avoids cross-engine handoff holes)
                v8 = [pv8.tile([128, 2, HW], f8, tag=f"v8{ct}", name=f"v8{ct}")
                      for ct in range(CT)]
                for ct in range(CT):
                    nc.sync.dma_start(
                        out=v8[ct],
                        in_=xq_d[b, ct * 128:(ct + 1) * 128, :, :],
                    )
                return vt, v8

            batches = [bb for _ in range(reps) for bb in range(B_PER_CORE)]
            # gamma first, then all loads, before any stores: the DMA
            # ring chains simultaneously-ready DMAs in scheduled order,
            # so late/slow DMAs in front of the chain stall everything.
            load_gamma()
            loads = [load_batch(b, bi) for bi, b in enumerate(batches)]

            for bi, b in enumerate(batches):
                vt, v8 = loads[bi]

                # ---- E: energy (fp16 operands, f32 accum), quarter-outer
                # so the first matmuls start as soon as xbar quarter 0 lands.
                ep = [pep.tile([128, C], f32, tag=f"ep{ct}", name=f"ep{ct}")
                      for ct in range(CT)]
                NG = len(CHUNKS)
                # chunk-outer for the head (E starts on the first xbar
                # chunk), then ct-outer for the last chunks so the four
                # row-tiles STOP staggered and their softmax chains
                # pipeline on DVE/ACT instead of piling up
                TAIL = 1

                def e_mm(g, ct, kk):
                    off = ct * 128
                    nc.tensor.matmul(
                        ep[ct][:, off:],
                        lhsT=vt[g][:, kk, off:off + 128],
                        rhs=vt[g][:, kk, off:],
                        start=(g == 0 and kk == 0),
                        stop=(g == NG - 1 and kk == CHUNKS[g] - 1),
                    )

                for g in range(NG - TAIL):
                    for ct in range(CT):
                        for kk in range(CHUNKS[g]):
                            e_mm(g, ct, kk)
                for ct in range(CT):
                    for g in range(NG - TAIL, NG):
                        for kk in range(CHUNKS[g]):
                            e_mm(g, ct, kk)
                # ---- S: fused softmax -> fp8 att (+ identity on diagonal)
                mirror_sb = {}
                a8s = [None] * CT

                def epilogue(ct):
                    off = ct * 128
                    # stash blocks that later rows mirror
                    for dst in range(ct + 1, CT):
                        sb = pmir.tile([128, 128], f32, tag=f"m{dst}{ct}")
                        nc.vector.tensor_copy(
                            out=sb, in_=ep[ct][:, dst * 128:(dst + 1) * 128],
                        )
                        mirror_sb[(dst, ct)] = sb
                    for dt in range(ct):
                        nc.tensor.transpose(
                            ep[ct][:, dt * 128:(dt + 1) * 128],
                            mirror_sb[(ct, dt)], identf,
                        )
                    mn = psmall.tile([128, 1], f32, tag="mn")
                    nc.vector.tensor_reduce(
                        out=mn, in_=ep[ct], axis=mybir.AxisListType.X,
                        op=mybir.AluOpType.min,
                    )
                    a16 = pa16.tile([128, C], f16, tag=f"a16_{ct % 2}")
                    ss = psmall.tile([128, 1], f32, tag="ss")
                    nc.scalar.activation(
                        out=a16, in_=ep[ct],
                        func=mybir.ActivationFunctionType.Exp,
                        bias=mn, scale=-1.0, accum_out=ss,
                    )
                    rg = psmall.tile([128, 1], f32, tag="rg")
                    nc.vector.reciprocal(out=rg, in_=ss)
                    nc.vector.tensor_mul(out=rg, in0=rg, in1=gam)
                    ag = pa8.tile([128, C], f16, tag=f"ag_{ct}")
                    nc.vector.tensor_scalar_mul(ag, a16, rg)
                    # identity fold: att' = gamma*att + I, so O emits
                    # gamma*att@v + v directly (on DVE right after the
                    # scale: avoids two cross-engine sem hops)
                    nc.vector.tensor_add(
                        out=ag[:, off:off + 128], in0=ag[:, off:off + 128],
                        in1=ident16,
                    )
                    a8s[ct] = ag

                # ---- A + O per row-tile; epilogue(ct+1) is emitted before
                # O(ct) so the softmax chain of the next tile overlaps the
                # current tile's output matmuls.
                def emit_A(ct):
                    # transpose in fp16 (fp8 PE transpose is rejected by
                    # walrus); quantize to fp8 in one PSUM->SBUF copy; the
                    # DoubleRow lhsT broadcasts it over the 2 k-slots
                    atp = pop.tile([128, CT, 128], f16, tag="op")
                    for dt in range(CT):
                        nc.tensor.transpose(
                            atp[:, dt, :],
                            a8s[ct][:, dt * 128:(dt + 1) * 128], ident16,
                        )
                    aT = paT.tile([128, CT, 128], f8, tag=f"aT{ct}")
                    nc.vector.tensor_copy(out=aT, in_=atp)
                    return aT

                def emit_O(ct, aT):
                    for np_ in range(NCH // 2):  # paired n-chunks per store
                        st = pstage.tile([128, 2, 512], f16, tag="st")
                        for half in range(2):
                            n = np_ * 2 + half
                            op = pop.tile([128, 512], f32, tag="op")
                            for dt in range(CT):
                                nc.tensor.matmul(
                                    op,
                                    lhsT=aT[:, dt, :].unsqueeze(1)
                                        .broadcast_to((128, 2, 128)),
                                    rhs=v8[dt][:, :, n * 512:(n + 1) * 512],
                                    start=(dt == 0),
                                    stop=(dt == CT - 1),
                                    perf_mode=mybir.MatmulPerfMode.DoubleRow,
                                )
                            if half == 0:
                                nc.vector.tensor_copy(out=st[:, half], in_=op)
                            else:
                                nc.scalar.copy(out=st[:, half], in_=op)
                        # single DMA queue (SP): cross-engine DMA ring
                        # chaining creates stall holes, so keep the ring
                        # order = SP program order
                        nc.sync.dma_start(
                            out=y_d[b, ct * 128:(ct + 1) * 128,
                                    np_ * 1024:(np_ + 1) * 1024],
                            in_=st,
                        )

                # A(ct) is emitted before epilogue(ct+1) so the attT dup
                # copies beat the next softmax chain into the DVE/ACT
                # queues (O(ct) start depends on the dup copies).
                epilogue(0)
                aTs = {}
                for ct in range(CT):
                    aTs[ct] = emit_A(ct)
                    if ct + 1 < CT:
                        epilogue(ct + 1)
                    emit_O(ct, aTs[ct])

    nc.compile()
    return nc


def host_prep(x: np.ndarray):
    """x (B, C, HW) f32 -> (x16 fp16, xq fp8-pair [B, C, 2, HW])."""
    x16 = x.astype(np.float16)
    hi = x.astype(F8NP)
    lo = (x - hi.astype(np.float32)).astype(F8NP)
    xq = np.stack([hi, lo], axis=2)
    return x16, xq


def kernel(x: np.ndarray, gamma: np.ndarray) -> np.ndarray:
    x = np.ascontiguousarray(np.asarray(x, dtype=np.float32))
    gamma = np.ascontiguousarray(np.asarray(gamma, dtype=np.float32))
    B, Cc, H, W = x.shape
    xv = x.reshape(B, Cc, H * W)
    x16, xq = host_prep(xv)

    if "nc" not in _CACHE:
        _CACHE["nc"] = _build_nc()
    nc = _CACHE["nc"]

    in_maps = [
        {
            "x16": x16[i * B_PER_CORE:(i + 1) * B_PER_CORE],
            "xq": xq[i * B_PER_CORE:(i + 1) * B_PER_CORE],
            "gamma": gamma,
        }
        for i in range(N_CORES)
    ]
    res = run_bass_kernel_spmd(nc, in_maps, list(range(N_CORES)))
    y = np.concatenate([res.results[i]["y"] for i in range(N_CORES)], axis=0)
    return y.astype(np.float32).reshape(B, Cc, H, W)
